# revision 3
# baseline (speedup 1.0000x reference)
"""Trainium2 Bass kernel for the BWSG ODE (nn_BWSGODE_naive_int).

Problem: single-trajectory 4-component quadratic Euler recurrence
(y0=[B,W,S,G,i], 10 params, num_steps sequential steps; output is the
full [T,5] trajectory).  The recurrence is inherently sequential, so the
kernel minimizes per-step latency on one NeuronCore and replicates the
same work across all 8 cores (pure SPMD; core 0's output is returned).

Per-step structure (state on SBUF partitions 0-3, column per step):
  d_s   = (E_{s-1} * state_{s-1}) * L_s        one DVE scalar_tensor_tensor
  col_s = state_{s-1} + d_s                    one DVE tensor_tensor
  E_s[1] = W_s (copy_predicated; other slots stay 1)
  L_{s+1} = L_s + ML^T @ d_s                   PE matmul, PSUM-accumulated
                                               (off the DVE critical path)
where L = ML^T @ [state;1] are the four linear forms of the ODE and
E = [1, W, 1, 1] supplies the extra W factor of dW = W^2*(...).  The
intervention mask only gates terms involving B, handled by a masked
coefficient matrix for the first n0 steps plus one PSUM rebase at the
phase switch — B stays frozen automatically because its linear form is 0.
"""
import sys

sys.path.insert(0, "/opt/trn_rl_repo")

import numpy as np

_NCORES = 8
_NC_CACHE = {}


def _make_mats(params):
    p = np.asarray(params, dtype=np.float32)
    ML1 = np.zeros((5, 4), dtype=np.float32)
    ML1[1, 0] = p[8]; ML1[2, 0] = p[8]; ML1[4, 0] = -p[9]
    ML1[0, 1] = -p[6]; ML1[2, 1] = p[5]; ML1[4, 1] = -p[7]
    ML1[0, 2] = -p[3]; ML1[1, 2] = -p[3]; ML1[3, 2] = p[2]; ML1[4, 2] = -p[4]
    ML1[2, 3] = -p[1]; ML1[3, 3] = -p[0]; ML1[4, 3] = p[0]
    ML0 = ML1.copy()
    ML0[:, 0] = 0.0
    ML0[0, 1] = 0.0
    ML0[0, 2] = 0.0
    return ML1, ML0


def _compute_n0(y0, T):
    """Number of leading masked steps, replicating the reference's f32 mask
    arithmetic: mask_j = (j >= 5.0 + i - 1.0) when i != 0."""
    f = np.float32
    i = f(np.asarray(y0, dtype=np.float32)[4])
    if i == f(0.0):
        return 0
    thresh = f(f(f(5.0) + i) - f(1.0))
    js = np.arange(1, T, dtype=np.float32)
    mask = js >= thresh
    if not mask.any():
        return T - 1
    return int(np.argmax(mask))


def _build_nc(T, n0):
    import concourse.bass as bass
    import concourse.mybir as mybir

    f32 = mybir.dt.float32
    A = mybir.AluOpType
    nc = bass.Bass()
    cin_d = nc.declare_dram_parameter("cin", [5, 16], f32, isOutput=False)
    out_d = nc.declare_dram_parameter("out", [5, T], f32, isOutput=True)

    traj = nc.sbuf_tensor([5, T], f32).__enter__()
    w5 = nc.sbuf_tensor([5, 16], f32).__enter__()
    dbuf = nc.sbuf_tensor([4, 2], f32).__enter__()
    Ebuf = nc.sbuf_tensor([4, 2], f32).__enter__()
    U = nc.psum_tensor([4, 1], f32).__enter__()

    nv0 = 3            # setup DVE ops
    per = 3            # stt + E-update + add per step
    n_dve = nv0 + per * (T - 1)
    s0 = n0 + 1        # first unmasked step (1-based)

    sel = w5[0:4, 9:10]
    invsel = w5[0:4, 10:11]
    ML1_5 = w5[0:5, 1:5]
    ML0_5 = w5[0:5, 5:9]
    ML1_4 = w5[0:4, 1:5]
    ML0_4 = w5[0:4, 5:9]

    def stt_count(s):
        return nv0 + per * (s - 1) + 1

    def tt_count(s):
        return nv0 + per * (s - 1) + 3

    with (
        nc.Block() as block,
        nc.semaphore("dma_sem") as dma_sem,
        nc.semaphore("vsem") as vsem,
        nc.semaphore("psem") as psem,
    ):

        @block.sync
        def _(sync):
            sync.dma_start(out=w5[0:5, 0:16], in_=cin_d[:, :]).then_inc(
                dma_sem, 16
            )
            sync.wait_ge(vsem, n_dve)
            sync.dma_start(out=out_d[:, :], in_=traj[0:5, 0:T]).then_inc(
                dma_sem, 16
            )

        @block.tensor
        def _(tensor):
            # U_1 = ML^T @ [state_0; 1]
            ins = tensor.matmul(
                U[0:4, 0:1], ML0_5 if 1 <= n0 else ML1_5,
                traj[0:5, 0:1], start=True, stop=True,
            )
            ins.wait_op(vsem, 2, "sem-ge")
            ins.then_inc(psem, 1)
            for s in range(2, T):
                if s == s0:
                    # phase switch: rebase L from the full unmasked matrix
                    ins = tensor.matmul(
                        U[0:4, 0:1], ML1_5, traj[0:5, s - 1 : s],
                        start=True, stop=True, skip_group_check=True,
                    )
                    ins.wait_op(vsem, tt_count(s - 1), "sem-ge")
                else:
                    ML4 = ML1_4 if s > s0 else ML0_4
                    pd = (s - 1) % 2
                    ins = tensor.matmul(
                        U[0:4, 0:1], ML4, dbuf[0:4, pd : pd + 1],
                        start=False, stop=False, skip_group_check=True,
                    )
                    ins.wait_op(vsem, stt_count(s - 1), "sem-ge")
                ins.then_inc(psem, 1)

        @block.vector
        def _(vector):
            k = 0

            def chain(emit, wait=None):
                nonlocal k
                ins = emit()
                if wait is not None:
                    ins.wait_op(*wait)
                ins.then_inc(vsem, 1)
                k += 1
                return ins

            # row 4 (and everything else) = 1.0; real rows overwritten below
            chain(lambda: vector.memset(traj[0:5, 0:T], 1.0))
            vector.wait_ge(dma_sem, 16)
            chain(lambda: vector.tensor_scalar_add(
                traj[0:5, 0:1], w5[0:5, 0:1], 0.0),
                wait=(vsem, 1, "sem-ge"))
            # E_0 = sel*state_0 + invsel = [1, W_0, 1, 1]
            chain(lambda: vector.tensor_scalar(
                out=Ebuf[0:4, 0:1], in0=traj[0:4, 0:1],
                scalar1=sel, scalar2=invsel, op0=A.mult, op1=A.add),
                wait=(vsem, 2, "sem-ge"))

            for s in range(1, T):
                st4 = traj[0:4, s - 1 : s]
                pd = s % 2
                vector.wait_ge(psem, s)
                # d = (L * state) * E
                chain(lambda: vector.scalar_tensor_tensor(
                    out=dbuf[0:4, pd : pd + 1], in0=Ebuf[0:4, 0:1],
                    scalar=st4, in1=U[0:4, 0:1],
                    op0=A.mult, op1=A.mult),
                    wait=(vsem, k, "sem-ge"))
                # state' = state + d
                chain(lambda: vector.tensor_tensor(
                    out=traj[0:4, s : s + 1], in0=st4,
                    in1=dbuf[0:4, pd : pd + 1],
                    op=A.add),
                    wait=(vsem, k, "sem-ge"))
                # E[1] <- new W (other slots preserved); int-bit mask
                chain(lambda: vector.copy_predicated(
                    out=Ebuf[0:4, 0:1],
                    mask=w5[0:4, 11:12].bitcast(mybir.dt.int32),
                    data=traj[0:4, s : s + 1]),
                    wait=(vsem, k, "sem-ge"))

    return nc


def _host_prepare(y0, params, T):
    y0 = np.asarray(y0, dtype=np.float32)
    params = np.asarray(params, dtype=np.float32)
    n0 = _compute_n0(y0, T)
    ML1, ML0 = _make_mats(params)
    cin = np.zeros((5, 16), dtype=np.float32)
    cin[0:4, 0] = y0[0:4]
    cin[4, 0] = 1.0
    cin[:, 1:5] = ML1
    cin[:, 5:9] = ML0
    cin[0:4, 9] = np.float32([0, 1, 0, 0])
    cin[0:4, 10] = np.float32([1, 0, 1, 1])
    cin[0:4, 11] = np.array([0, 1, 0, 0], np.int32).view(np.float32)
    return n0, cin


def _host_finish(raw_out, y0, T):
    a = np.asarray(raw_out, dtype=np.float32).reshape(5, T)
    out = np.empty((T, 5), dtype=np.float32)
    out[:, 0:4] = a[0:4, :].T
    out[:, 4] = np.float32(np.asarray(y0, dtype=np.float32)[4])
    return out


def kernel(y0, params, num_steps):
    y0 = np.asarray(y0, dtype=np.float32)
    params = np.asarray(params, dtype=np.float32)
    T = int(num_steps)

    if T <= 1:
        out = np.empty((max(T, 0), 5), dtype=np.float32)
        if T >= 1:
            out[0, 0:4] = y0[0:4]
            out[0, 4] = y0[4]
        return out

    n0, cin = _host_prepare(y0, params, T)

    key = (T, n0)
    if key not in _NC_CACHE:
        _NC_CACHE[key] = _build_nc(T, n0)
    nc = _NC_CACHE[key]

    from concourse.bass_utils import run_bass_kernel_spmd

    in_maps = [{"cin": cin} for _ in range(_NCORES)]
    res = run_bass_kernel_spmd(nc, in_maps, list(range(_NCORES)))
    return _host_finish(res.results[0]["out"], y0, T)
